# revision 22
# baseline (speedup 1.0000x reference)
"""Trainium2 Bass kernel for nn_BertLexer (weighted layer mix + ragged segment-mean).

Computation (reference):
    w   = softmax(layer_weights)                       # (L,)
    sub = gamma * einsum('l,lbsf->bsf', w, hidden)     # (B,S,F)
    out[b,w,:] = mean over {s : word_ids[b,s]==w} of sub[b,s,:]   (w >= 1)
    out[b,0,:] = mean over all s of sub[b,s,:]

Strategy (8 NeuronCores, data-parallel over B):
  - Each core gets B/8 = 4 sentences.  Per-core HBM traffic ~28.4 MB
    (25.2 hid in + 0.26 aux + 3.2 out) ~ 79 us at 358 GB/s.
  - Layer mix on DVE: 3 scalar_tensor_tensor ops per 128x768 chunk via
    ratio folding over weight-sorted layers (a<=b<=c<=d by softmax weight):
    t1 = h_a*(w_a/w_d) + h_d ; t2 = h_b*(w_b/w_c) + h_c ;
    sub = t2*(w_c/w_d) + t1, and the segment matrix absorbs w_d*gamma.
  - Segment matrix M[s, w] = w_d*gamma/count_w for s in word w's span
    (M[s,0] = w_d*gamma/S dense) is built ON DEVICE by GpSimd from a tiny
    aux table (ids + per-position reciprocal):
      M = (iota_w == ids[s]) * v[s]  then  M[:,0] = scale/S.
    This replaces the 2.1 MB host-built matrix DMA of the old version.
  - Segment mean as f32r matmuls contracting over s, PSUM-accumulated
    over the 4 s-chunks; w tiled 128+128+1, f split 2x384 (PSUM bank).
  - Loads: per (b,c,l) 393KB; layers 0,1 on the SP HWDGE ring, layers
    2,3 on the ACT ring.  Stores: sentence 0 inline on ACT; sentences
    1-3 issued after all loads on the SP ring so they queue behind the
    loads and fill the end-of-kernel compute bubble.
"""

import numpy as np

L, B, S, F = 4, 32, 512, 768
W_MAX = 256
NW = W_MAX + 1  # 257
NCORES = 8
NB = B // NCORES  # sentences per core
P = 128
SC = S // P  # s-chunks per sentence

_module_cache: dict = {}

AUX_W = 128  # padded aux row (f32 per partition); first NB*SC*2 used


def _build_module(r0: float, r1: float, r2: float, col0: float, order):
    import concourse.bacc as bacc
    import concourse.bass as bass
    import concourse.mybir as mybir
    import concourse.tile as tile

    f32 = mybir.dt.float32
    bf16 = mybir.dt.bfloat16
    mult = mybir.AluOpType.mult
    add = mybir.AluOpType.add
    is_eq = mybir.AluOpType.is_equal

    nc = bacc.Bacc(
        "TRN2", target_bir_lowering=False, debug=False, num_devices=NCORES
    )
    hid = nc.dram_tensor("hid", (L, NB, S, F), f32, kind="ExternalInput").ap()
    aux = nc.dram_tensor("aux", (P, AUX_W), f32, kind="ExternalInput").ap()
    out = nc.dram_tensor("out", (NB, NW, F), f32, kind="ExternalOutput").ap()

    uniform = r0 == 1.0 and r1 == 1.0 and r2 == 1.0
    wtiles = [(0, 128), (128, 256), (256, 257)]
    fsplits = [(0, 384), (384, 768)]
    ia, ib, ic, id_ = order

    with tile.TileContext(nc) as tc:
        with (
            tc.tile_pool(name="const", bufs=1) as cpool,
            tc.tile_pool(name="h", bufs=24) as hpool,
            tc.tile_pool(name="t", bufs=4) as tpool,
            tc.tile_pool(name="sub", bufs=6) as spool,
            tc.tile_pool(name="m", bufs=NB * SC) as mpool,
            tc.tile_pool(name="o", bufs=NB * len(wtiles)) as opool,
            tc.tile_pool(name="ps", bufs=4, space=bass.MemorySpace.PSUM) as pspool,
        ):
            # tiny aux load first on the SP ring; iota rows 0..256 built once
            aux_t = cpool.tile([P, AUX_W], f32, name="aux")
            nc.sync.dma_start(aux_t[:], aux)
            iota_t = cpool.tile([P, NW], f32, name="iota")
            nc.gpsimd.iota(
                iota_t[:],
                pattern=[[1, NW]],
                base=0,
                channel_multiplier=0,
                allow_small_or_imprecise_dtypes=True,
            )

            def build_m(b, c):
                """segment matrix M[s, w] = (iota_w == ids[s]) * v[s]; col0."""
                j = (b * SC + c) * 2
                mt = mpool.tile([P, NW], bf16, tag="m", name=f"m{b}_{c}")
                nc.gpsimd.memset(mt[:, 0:1], col0)
                nc.vector.tensor_scalar(
                    mt[:, 1:NW],
                    iota_t[:, 1:NW],
                    aux_t[:, j : j + 1],
                    aux_t[:, j + 1 : j + 2],
                    op0=is_eq,
                    op1=mult,
                )
                return mt

            def mix(dst, h, f0, f1):
                """dst[:, f0:f1] (bf16) = folded weighted layer sum."""
                t1 = tpool.tile([P, f1 - f0], f32, tag="t")
                nc.vector.scalar_tensor_tensor(
                    t1[:], h[ia][:, f0:f1], float(r0), h[id_][:, f0:f1],
                    op0=mult, op1=add,
                )
                t2 = tpool.tile([P, f1 - f0], f32, tag="t")
                nc.vector.scalar_tensor_tensor(
                    t2[:], h[ib][:, f0:f1], float(r1), h[ic][:, f0:f1],
                    op0=mult, op1=add,
                )
                nc.vector.scalar_tensor_tensor(
                    dst[:], t2[:], float(r2), t1[:], op0=mult, op1=add
                )

            def matmuls(ps_tiles, mc, sub_aps, c):
                for t, (w0, w1) in enumerate(wtiles):
                    msz = w1 - w0
                    for fi in range(2):
                        nc.tensor.matmul(
                            ps_tiles[t][0:msz, fi, 0:384],
                            mc[:, w0:w1],
                            sub_aps[fi],
                            start=(c == 0),
                            stop=(c == SC - 1),
                        )

            held_stores = []  # (b, obs) deferred to after all loads
            for b in range(NB):
                last = b == NB - 1
                # build this sentence's segment matrices ahead of its chunks
                # (fills DVE idle; keeps the M work off the critical tail)
                mcs = [build_m(b, c) for c in range(SC)]
                ps_tiles = [
                    pspool.tile([P, 2, 512], f32, tag="ps", name=f"ps{b}_{t}")
                    for t in range(len(wtiles))
                ]
                for c in range(SC):
                    h = []
                    for l in range(L):
                        ht = hpool.tile(
                            [P, F], f32, tag="h", name=f"h{b}_{c}_{l}"
                        )
                        if b == 0 and c == 0:
                            # the ACT engine's preamble delays its first
                            # trigger ~5us; feed chunk 0 entirely from SP
                            eng = nc.sync
                        else:
                            eng = nc.sync if l < 2 else nc.scalar
                        eng.dma_start(ht[:], hid[l, b, c * P : (c + 1) * P, :])
                        h.append(ht)
                    if last:
                        # half-chunk mixes shorten the after-last-load tail
                        subs = []
                        for fi, (f0, f1) in enumerate(fsplits):
                            sb = spool.tile([P, 384], bf16, tag="subh")
                            mix(sb, h, f0, f1)
                            subs.append(sb[:])
                        matmuls(ps_tiles, mcs[c], subs, c)
                    else:
                        sb = spool.tile([P, F], bf16, tag="sub")
                        mix(sb, h, 0, F)
                        matmuls(
                            ps_tiles,
                            mcs[c],
                            [sb[:, f0:f1] for (f0, f1) in fsplits],
                            c,
                        )
                obs = []
                for t, (w0, w1) in enumerate(wtiles):
                    msz = w1 - w0
                    ob = opool.tile([P, F], f32, tag="o", name=f"o{b}_{t}")
                    nc.scalar.copy(ob[0:msz, :], ps_tiles[t][0:msz, :, 0:384])
                    obs.append((w0, w1, ob))
                if not last:
                    for w0, w1, ob in obs:
                        nc.scalar.dma_start(out[b, w0:w1, :], ob[0 : w1 - w0, :])
                else:
                    held_stores.append((b, obs))
            b_last = NB - 1
            # deferred stores: queue behind all loads on the SP ring so they
            # drain during (and after) the final sentence's compute.  The
            # final sentence ships in f-halves right behind its copies.
            for b, obs in held_stores:
                for w0, w1, ob in obs:
                    if b == b_last:
                        nc.sync.dma_start(
                            out[b, w0:w1, 0:384], ob[0 : w1 - w0, 0:384]
                        )
                        nc.sync.dma_start(
                            out[b, w0:w1, 384:768], ob[0 : w1 - w0, 384:768]
                        )
                    else:
                        nc.sync.dma_start(out[b, w0:w1, :], ob[0 : w1 - w0, :])

    nc.compile()
    return nc


def _prepare(hidden_states, layer_weights, gamma, word_ids):
    """Host-side prep: softmax ratios + per-position recip table + shards."""
    hidden_states = np.ascontiguousarray(hidden_states, dtype=np.float32)
    lw = np.asarray(layer_weights, dtype=np.float64)
    g = float(np.asarray(gamma, dtype=np.float64).reshape(-1)[0])
    ids = np.asarray(word_ids)

    e = np.exp(lw - lw.max())
    w = e / e.sum()  # softmax, float64
    # pair layers sorted by weight so every folded ratio is <= 1:
    #   sub*w[d] = w[a]h[a] + w[b]h[b] + w[c]h[c] + w[d]h[d]
    order = tuple(int(i) for i in np.argsort(w))
    ia, ib, ic, id_ = order
    r0 = float(w[ia] / w[id_])
    r1 = float(w[ib] / w[ic]) if w[ic] > 0 else 0.0
    r2 = float(w[ic] / w[id_])
    scale = float(w[id_] * g)  # absorbed into M
    col0 = float(np.float32(scale / S))

    # per-position aux: ids (as f32) and v[s] = scale/count[ids[s]]
    vmat = np.zeros((B, S), dtype=np.float64)
    for b in range(B):
        counts = np.bincount(ids[b], minlength=NW).astype(np.float64)
        recip = np.zeros(NW, dtype=np.float64)
        nz = counts > 0
        recip[nz] = scale / counts[nz]
        recip[0] = scale / S  # pad rows get overwritten by the col-0 memset
        vmat[b] = recip[ids[b]]
    ids_f = ids.astype(np.float32).reshape(B, SC, P)
    v_f = vmat.astype(np.float32).reshape(B, SC, P)

    in_maps = []
    for i in range(NCORES):
        bs = slice(i * NB, (i + 1) * NB)
        aux_np = np.zeros((P, AUX_W), dtype=np.float32)  # cast to bf16 below
        idsc = ids_f[bs]  # (NB, SC, P)
        vc = v_f[bs]
        for b in range(NB):
            for c in range(SC):
                j = (b * SC + c) * 2
                aux_np[:, j] = idsc[b, c]
                aux_np[:, j + 1] = vc[b, c]
        in_maps.append(
            {
                "hid": np.ascontiguousarray(hidden_states[:, bs]),
                "aux": aux_np,
            }
        )
    return (r0, r1, r2, col0, order), in_maps


def _run(inputs: dict, trace: bool = False):
    from concourse.bass_utils import run_bass_kernel_spmd

    params, in_maps = _prepare(**inputs)
    if params not in _module_cache:
        _module_cache[params] = _build_module(*params)
    nc = _module_cache[params]

    res = run_bass_kernel_spmd(
        nc, in_maps, core_ids=list(range(NCORES)), trace=trace
    )
    out = np.concatenate([r["out"] for r in res.results], axis=0)
    return out, res


def kernel(**inputs) -> np.ndarray:
    out, _ = _run(inputs, trace=False)
    return out
